# revision 3
# baseline (speedup 1.0000x reference)
"""CrossAttentionFusion Trainium2 kernel.

Full inputs -> shard (batch x query-half) over 8 NeuronCores -> full output.

Per core (batch b = core//2, query half h = core%2, NH=2048 queries):
  K = k_w @ x2 + k_b              [C, N]   (all 4096 keys, c on partitions)
  Q = q_w @ x1[:, half] + q_b     [C, NH]
  VT = x2^T @ v_w^T               [N, C]   (m on partitions; v_b folded into Bc)
  gate = sigmoid(gate_w . [x1;x2] + gate_b)   [1, NH]
  per 512-query block:
    L[m, n] = K^T Q               (32 m-tiles, fp32r matmuls)
    E = exp(L / 16)               (ACT, no max subtraction: logits are O(1))
    S[n] = sum_m E[m, n]          (ones-matmul chain)
    F_un[c, n] = sum_m V[c, m] E[m, n]
    M1 = proj_w @ F_un
    out = x1 + gate * relu(M1 * G * (1/S) + Bc)
  where G = gamma * rsqrt(var+eps), Bc = beta + (proj_b + proj_w@v_b - mean)*G.

Everything on the PE runs in float32r (measured ~2e-4 rel err, full-rate).
"""
from contextlib import ExitStack

import numpy as np

import concourse.bass as bass
import concourse.mybir as mybir
import concourse.tile as tile
from concourse import bacc
from concourse.bass_utils import run_bass_kernel_spmd

F32 = mybir.dt.float32
F32R = mybir.dt.float32r
AF = mybir.ActivationFunctionType
OP = mybir.AluOpType

B, C, H, W = 4, 256, 64, 64
N = H * W            # 4096
NCORES = 8
NH = N // 2          # 2048 queries per core
NBLK = 512           # query block
NBLOCKS = NH // NBLK
MT = N // 128        # 32 m-tiles
EPS = 1e-5
SCALE = float(C) ** -0.5


def build():
    nc = bacc.Bacc("TRN2", target_bir_lowering=False, debug=False,
                   num_devices=NCORES)
    x1r_d = nc.dram_tensor("x1r", [C, NH], F32R, kind="ExternalInput")
    x1f_d = nc.dram_tensor("x1f", [C, NH], F32, kind="ExternalInput")
    x2r_d = nc.dram_tensor("x2r", [C, N], F32R, kind="ExternalInput")
    wm_d = nc.dram_tensor("wmat", [C, 4 * C], F32R, kind="ExternalInput")
    gw_d = nc.dram_tensor("gw", [C, 2], F32R, kind="ExternalInput")
    vec_d = nc.dram_tensor("vecs", [C, 4], F32, kind="ExternalInput")
    gb_d = nc.dram_tensor("gateb", [1, 1], F32, kind="ExternalInput")
    out_d = nc.dram_tensor("out", [C, NH], F32, kind="ExternalOutput")

    with tile.TileContext(nc) as tc, ExitStack() as ctx:
        pers = ctx.enter_context(tc.tile_pool(name="pers", bufs=1))
        work = ctx.enter_context(tc.tile_pool(name="work", bufs=2))
        psum = ctx.enter_context(tc.tile_pool(name="psum", bufs=1, space="PSUM"))

        # ---- persistent tiles ----
        wm = [pers.tile([128, 4 * C], F32R, tag=f"wm{ci}", name=f"wm{ci}") for ci in range(2)]
        gw = [pers.tile([128, 2], F32R, tag=f"gw{ci}", name=f"gw{ci}") for ci in range(2)]
        vec = [pers.tile([128, 4], F32, tag=f"vec{ci}", name=f"vec{ci}") for ci in range(2)]
        gb = pers.tile([1, 1], F32, tag="gb", name="gb")
        Kt = [pers.tile([128, N], F32R, tag=f"Kt{co}", name=f"Kt{co}") for co in range(2)]
        Qt = [pers.tile([128, NH], F32R, tag=f"Qt{co}", name=f"Qt{co}") for co in range(2)]
        VT = pers.tile([128, MT * C], F32R, tag="VT", name="VT")
        grow = pers.tile([1, NH], F32R, tag="grow", name="grow")
        ones_f = pers.tile([128, 1], F32, tag="ones_f", name="ones_f")
        ones_f2 = pers.tile([1, 128], F32, tag="ones_f2", name="ones_f2")
        ones_c = pers.tile([128, 1], F32R, tag="ones_c", name="ones_c")     # S-chain lhsT [K=128, M=1]
        ones_k1 = pers.tile([1, 128], F32R, tag="ones_k1", name="ones_k1")   # bcast lhsT  [K=1, M=128]

        with nc.named_scope("pre"):
            for ci in range(2):
                cs = slice(ci * 128, (ci + 1) * 128)
                nc.sync.dma_start(wm[ci][:], wm_d[cs, :])
                nc.sync.dma_start(gw[ci][:], gw_d[cs, :])
                nc.sync.dma_start(vec[ci][:], vec_d[cs, :])
            nc.sync.dma_start(gb[:], gb_d[:])
            nc.vector.memset(ones_f[:], 1.0)
            nc.vector.tensor_copy(ones_c[:], ones_f[:])
            nc.vector.memset(ones_f2[:], 1.0)
            nc.vector.tensor_copy(ones_k1[:], ones_f2[:])

            with tc.tile_pool(name="xin", bufs=1) as xin:
                x2r = [xin.tile([128, N], F32R, tag=f"x2r{ci}", name=f"x2r{ci}") for ci in range(2)]
                x1r = [xin.tile([128, NH], F32R, tag=f"x1r{ci}", name=f"x1r{ci}") for ci in range(2)]
                for ci in range(2):
                    cs = slice(ci * 128, (ci + 1) * 128)
                    nc.sync.dma_start(x2r[ci][:], x2r_d[cs, :])
                    nc.sync.dma_start(x1r[ci][:], x1r_d[cs, :])

                # K projection: K[co, m] = sum_ci k_wT[ci, co] x2[ci, m] (+ k_b)
                for co in range(2):
                    for nch in range(N // NBLK):
                        ns = slice(nch * NBLK, (nch + 1) * NBLK)
                        kp = psum.tile([128, NBLK], F32, tag="acc", name="acc", bufs=3)
                        for ci in range(2):
                            nc.tensor.matmul(
                                kp[:], wm[ci][:, C + co * 128: C + (co + 1) * 128],
                                x2r[ci][:, ns], start=(ci == 0), stop=(ci == 1))
                        nc.vector.tensor_scalar_add(Kt[co][:, ns], kp[:],
                                                    vec[co][:, 1:2])
                # Q projection
                for co in range(2):
                    for nch in range(NH // NBLK):
                        ns = slice(nch * NBLK, (nch + 1) * NBLK)
                        qp = psum.tile([128, NBLK], F32, tag="acc", name="acc", bufs=3)
                        for ci in range(2):
                            nc.tensor.matmul(
                                qp[:], wm[ci][:, co * 128:(co + 1) * 128],
                                x1r[ci][:, ns], start=(ci == 0), stop=(ci == 1))
                        nc.vector.tensor_scalar_add(Qt[co][:, ns], qp[:],
                                                    vec[co][:, 0:1])
                # VT projection: VT[m, co] = sum_ci x2[ci, m] v_wT[ci, co]
                for mt in range(MT):
                    ms = slice(mt * 128, (mt + 1) * 128)
                    vp = psum.tile([128, C], F32, tag="acc", name="acc", bufs=3)
                    for ci in range(2):
                        nc.tensor.matmul(vp[:], x2r[ci][:, ms],
                                         wm[ci][:, 2 * C:3 * C],
                                         start=(ci == 0), stop=(ci == 1))
                    nc.vector.tensor_copy(VT[:, mt * C:(mt + 1) * C], vp[:])
                # gate row: x2 columns are pre-permuted so query pixels = 0..NH
                for blk in range(NBLOCKS):
                    ns = slice(blk * NBLK, (blk + 1) * NBLK)
                    gp = psum.tile([1, NBLK], F32, tag="s", name="s", bufs=1)
                    for ci in range(2):
                        nc.tensor.matmul(gp[:], gw[ci][:, 0:1], x1r[ci][:, ns],
                                         start=(ci == 0), stop=False)
                    for ci in range(2):
                        nc.tensor.matmul(gp[:], gw[ci][:, 1:2], x2r[ci][:, ns],
                                         start=False, stop=(ci == 1))
                    nc.scalar.activation(grow[:, ns], gp[:], AF.Sigmoid,
                                         bias=gb[:])

        # ---- main attention loop over query blocks ----
        epool = ctx.enter_context(tc.tile_pool(name="epool", bufs=1))
        for blk in range(NBLOCKS):
            ns = slice(blk * NBLK, (blk + 1) * NBLK)
            E = epool.tile([128, MT * NBLK], F32R, tag="E", name="E")
            with nc.named_scope(f"logits{blk}"):
                for mt2 in range(MT // 2):
                    lp = psum.tile([128, 2 * NBLK], F32, tag="L", name="L", bufs=2)
                    for sub in range(2):
                        mt = 2 * mt2 + sub
                        msl = slice(mt * 128, (mt + 1) * 128)
                        for ci in range(2):
                            nc.tensor.matmul(
                                lp[:, sub * NBLK:(sub + 1) * NBLK],
                                Kt[ci][:, msl], Qt[ci][:, ns],
                                start=(ci == 0), stop=(ci == 1))
                    nc.scalar.activation(
                        E[:, mt2 * 2 * NBLK:(mt2 + 1) * 2 * NBLK], lp[:],
                        AF.Exp, scale=SCALE)
            with nc.named_scope(f"fusion{blk}"):
                sp = psum.tile([1, NBLK], F32, tag="s", name="s", bufs=1)
                fp = [psum.tile([128, NBLK], F32, tag="acc", name="acc", bufs=3)
                      for _ in range(2)]
                for mt in range(MT):
                    es = slice(mt * NBLK, (mt + 1) * NBLK)
                    for co in range(2):
                        nc.tensor.matmul(
                            fp[co][:], VT[:, mt * C + co * 128: mt * C + (co + 1) * 128],
                            E[:, es], start=(mt == 0), stop=(mt == MT - 1))
                    nc.tensor.matmul(sp[:], ones_c[:], E[:, es],
                                     start=(mt == 0), stop=(mt == MT - 1))
                invs_f = work.tile([1, NBLK], F32, tag="invs_f", name="invs_f")
                nc.vector.reciprocal(invs_f[:], sp[:])
                invs_r = work.tile([1, NBLK], F32R, tag="invs_r", name="invs_r")
                nc.vector.tensor_copy(invs_r[:], invs_f[:])
                # broadcast 1/S and gate across partitions via K=1 matmuls
                bc1 = psum.tile([128, NBLK], F32, tag="acc", name="acc", bufs=3)
                nc.tensor.matmul(bc1[:], ones_k1[:], invs_r[:])
                invs_b = work.tile([128, NBLK], F32, tag="invs_b", name="invs_b")
                nc.vector.tensor_copy(invs_b[:], bc1[:])
                bc2 = psum.tile([128, NBLK], F32, tag="acc", name="acc", bufs=3)
                nc.tensor.matmul(bc2[:], ones_k1[:], grow[:, ns])
                gate_b = work.tile([128, NBLK], F32, tag="gate_b", name="gate_b")
                nc.vector.tensor_copy(gate_b[:], bc2[:])
                Fs = [work.tile([128, NBLK], F32R, tag=f"Fs{co}", name=f"Fs{co}")
                      for co in range(2)]
                for co in range(2):
                    nc.vector.tensor_copy(Fs[co][:], fp[co][:])
            with nc.named_scope(f"post{blk}"):
                for co in range(2):
                    cs = slice(co * 128, (co + 1) * 128)
                    mp = psum.tile([128, NBLK], F32, tag="acc", name="acc", bufs=3)
                    for ci in range(2):
                        nc.tensor.matmul(
                            mp[:], wm[ci][:, 3 * C + co * 128: 3 * C + (co + 1) * 128],
                            Fs[ci][:], start=(ci == 0), stop=(ci == 1))
                    x1t = work.tile([128, NBLK], F32, tag="x1t", name="x1t")
                    nc.sync.dma_start(x1t[:], x1f_d[cs, ns])
                    t1 = work.tile([128, NBLK], F32, tag="t1", name="t1")
                    nc.vector.scalar_tensor_tensor(
                        t1[:], mp[:], vec[co][:, 2:3], invs_b[:],
                        op0=OP.mult, op1=OP.mult)
                    r = work.tile([128, NBLK], F32, tag="r", name="r")
                    nc.scalar.activation(r[:], t1[:], AF.Relu,
                                         bias=vec[co][:, 3:4])
                    rg = work.tile([128, NBLK], F32, tag="rg", name="rg")
                    nc.vector.tensor_mul(rg[:], r[:], gate_b[:])
                    ot = work.tile([128, NBLK], F32, tag="ot", name="ot")
                    nc.vector.tensor_add(ot[:], rg[:], x1t[:])
                    nc.sync.dma_start(out_d[cs, ns], ot[:])
    nc.compile()
    return nc


_NC = None


def _get_nc():
    global _NC
    if _NC is None:
        _NC = build()
    return _NC


def kernel(**inputs):
    x1 = np.ascontiguousarray(np.asarray(inputs["x1"], dtype=np.float32)).reshape(B, C, N)
    x2 = np.ascontiguousarray(np.asarray(inputs["x2"], dtype=np.float32)).reshape(B, C, N)
    q_w = np.asarray(inputs["q_w"], np.float32)
    k_w = np.asarray(inputs["k_w"], np.float32)
    v_w = np.asarray(inputs["v_w"], np.float32)
    p_w = np.asarray(inputs["proj_w"], np.float32)
    q_b = np.asarray(inputs["q_b"], np.float32)
    k_b = np.asarray(inputs["k_b"], np.float32)
    v_b = np.asarray(inputs["v_b"], np.float32)
    p_b = np.asarray(inputs["proj_b"], np.float32)
    gamma = np.asarray(inputs["bn_gamma"], np.float32)
    beta = np.asarray(inputs["bn_beta"], np.float32)
    mean = np.asarray(inputs["bn_mean"], np.float32)
    var = np.asarray(inputs["bn_var"], np.float32)
    gate_w = np.asarray(inputs["gate_w"], np.float32)
    gate_b = np.asarray(inputs["gate_b"], np.float32)

    wmat = np.ascontiguousarray(
        np.concatenate([q_w.T, k_w.T, v_w.T, p_w.T], axis=1))  # [C, 4C]
    gw = np.ascontiguousarray(np.stack([gate_w[0, :C], gate_w[0, C:]], axis=1))
    G = gamma / np.sqrt(var + EPS)
    Bc = beta + (p_b + p_w @ v_b - mean) * G
    vecs = np.ascontiguousarray(np.stack([q_b, k_b, G, Bc], axis=1))  # [C, 4]
    gb = gate_b.reshape(1, 1)

    in_maps = []
    for core in range(NCORES):
        b, half = divmod(core, 2)
        hq = slice(half * NH, (half + 1) * NH)
        ho = slice((1 - half) * NH, (2 - half) * NH)
        x1q = np.ascontiguousarray(x1[b][:, hq])
        x2p = np.ascontiguousarray(np.concatenate([x2[b][:, hq], x2[b][:, ho]],
                                                  axis=1))
        in_maps.append({
            "x1r": x1q, "x1f": x1q, "x2r": x2p,
            "wmat": wmat, "gw": gw, "vecs": vecs, "gateb": gb,
        })

    nc = _get_nc()
    res = run_bass_kernel_spmd(nc, in_maps, core_ids=list(range(NCORES)))
    out = np.empty((B, C, N), np.float32)
    for core in range(NCORES):
        b, half = divmod(core, 2)
        out[b, :, half * NH:(half + 1) * NH] = res.results[core]["out"]
    return out.reshape(B, C, H, W)


# revision 4
# speedup vs baseline: 1.0067x; 1.0067x over previous
"""CrossAttentionFusion Trainium2 kernel.

Full inputs -> shard (batch x query-half) over 8 NeuronCores -> full output.

Per core (batch b = core//2, query half h = core%2, NH=2048 queries):
  K = k_w @ x2 + k_b              [C, N]   (all 4096 keys, c on partitions)
  Q = q_w @ x1[:, half] + q_b     [C, NH]
  VT = x2^T @ v_w^T               [N, C]   (m on partitions; v_b folded into Bc)
  gate = sigmoid(gate_w . [x1;x2] + gate_b)   [1, NH]
  per 512-query block:
    L[m, n] = K^T Q               (32 m-tiles, fp32r matmuls)
    E = exp(L / 16)               (ACT, no max subtraction: logits are O(1))
    S[n] = sum_m E[m, n]          (ones-matmul chain)
    F_un[c, n] = sum_m V[c, m] E[m, n]
    M1 = proj_w @ F_un
    out = x1 + gate * relu(M1 * G * (1/S) + Bc)
  where G = gamma * rsqrt(var+eps), Bc = beta + (proj_b + proj_w@v_b - mean)*G.

Everything on the PE runs in float32r (measured ~2e-4 rel err, full-rate).
"""
from contextlib import ExitStack

import numpy as np

import concourse.bass as bass
import concourse.mybir as mybir
import concourse.tile as tile
from concourse import bacc
from concourse.bass_utils import run_bass_kernel_spmd

F32 = mybir.dt.float32
F32R = mybir.dt.float32r
AF = mybir.ActivationFunctionType
OP = mybir.AluOpType

B, C, H, W = 4, 256, 64, 64
N = H * W            # 4096
NCORES = 8
NH = N // 2          # 2048 queries per core
NBLK = 512           # query block
NBLOCKS = NH // NBLK
MT = N // 128        # 32 m-tiles
EPS = 1e-5
SCALE = float(C) ** -0.5


def build():
    nc = bacc.Bacc("TRN2", target_bir_lowering=False, debug=False,
                   num_devices=NCORES)
    x1r_d = nc.dram_tensor("x1r", [C, NH], F32R, kind="ExternalInput")
    x1f_d = nc.dram_tensor("x1f", [C, NH], F32, kind="ExternalInput")
    x2r_d = nc.dram_tensor("x2r", [C, N], F32R, kind="ExternalInput")
    wm_d = nc.dram_tensor("wmat", [C, 4 * C], F32R, kind="ExternalInput")
    gw_d = nc.dram_tensor("gw", [C, 2], F32R, kind="ExternalInput")
    vec_d = nc.dram_tensor("vecs", [C, 4], F32, kind="ExternalInput")
    gb_d = nc.dram_tensor("gateb", [1, 1], F32, kind="ExternalInput")
    out_d = nc.dram_tensor("out", [C, NH], F32, kind="ExternalOutput")

    with tile.TileContext(nc) as tc, ExitStack() as ctx:
        pers = ctx.enter_context(tc.tile_pool(name="pers", bufs=1))
        work = ctx.enter_context(tc.tile_pool(name="work", bufs=2))
        psum = ctx.enter_context(tc.tile_pool(name="psum", bufs=1, space="PSUM"))

        # ---- persistent tiles ----
        wm = [pers.tile([128, 4 * C], F32R, tag=f"wm{ci}", name=f"wm{ci}") for ci in range(2)]
        gw = [pers.tile([128, 2], F32R, tag=f"gw{ci}", name=f"gw{ci}") for ci in range(2)]
        vec = [pers.tile([128, 4], F32, tag=f"vec{ci}", name=f"vec{ci}") for ci in range(2)]
        gb = pers.tile([1, 1], F32, tag="gb", name="gb")
        Kt = [pers.tile([128, N], F32R, tag=f"Kt{co}", name=f"Kt{co}") for co in range(2)]
        Qt = [pers.tile([128, NH], F32R, tag=f"Qt{co}", name=f"Qt{co}") for co in range(2)]
        VT = pers.tile([128, MT * C], F32R, tag="VT", name="VT")
        grow = pers.tile([1, NH], F32R, tag="grow", name="grow")
        ones_f = pers.tile([128, 1], F32, tag="ones_f", name="ones_f")
        ones_f2 = pers.tile([1, 128], F32, tag="ones_f2", name="ones_f2")
        ones_c = pers.tile([128, 1], F32R, tag="ones_c", name="ones_c")     # S-chain lhsT [K=128, M=1]
        ones_k1 = pers.tile([1, 128], F32R, tag="ones_k1", name="ones_k1")   # bcast lhsT  [K=1, M=128]

        with nc.named_scope("pre"):
            for ci in range(2):
                cs = slice(ci * 128, (ci + 1) * 128)
                nc.sync.dma_start(wm[ci][:], wm_d[cs, :])
                nc.sync.dma_start(gw[ci][:], gw_d[cs, :])
                nc.sync.dma_start(vec[ci][:], vec_d[cs, :])
            nc.sync.dma_start(gb[:], gb_d[:])
            nc.vector.memset(ones_f[:], 1.0)
            nc.vector.tensor_copy(ones_c[:], ones_f[:])
            nc.vector.memset(ones_f2[:], 1.0)
            nc.vector.tensor_copy(ones_k1[:], ones_f2[:])

            with tc.tile_pool(name="xin", bufs=1) as xin:
                x2r = [xin.tile([128, N], F32R, tag=f"x2r{ci}", name=f"x2r{ci}") for ci in range(2)]
                x1r = [xin.tile([128, NH], F32R, tag=f"x1r{ci}", name=f"x1r{ci}") for ci in range(2)]
                CH = 1024
                for ch in range(N // CH):
                    chs = slice(ch * CH, (ch + 1) * CH)
                    nc.sync.dma_start(x2r[0][:, chs], x2r_d[0:128, chs])
                    nc.gpsimd.dma_start(x2r[1][:, chs], x2r_d[128:256, chs])
                for ch in range(NH // CH):
                    chs = slice(ch * CH, (ch + 1) * CH)
                    nc.sync.dma_start(x1r[0][:, chs], x1r_d[0:128, chs])
                    nc.gpsimd.dma_start(x1r[1][:, chs], x1r_d[128:256, chs])

                # K projection: K[co, m] = sum_ci k_wT[ci, co] x2[ci, m] (+ k_b)
                for co in range(2):
                    for nch in range(N // NBLK):
                        ns = slice(nch * NBLK, (nch + 1) * NBLK)
                        kp = psum.tile([128, NBLK], F32, tag="acc", name="acc", bufs=3)
                        for ci in range(2):
                            nc.tensor.matmul(
                                kp[:], wm[ci][:, C + co * 128: C + (co + 1) * 128],
                                x2r[ci][:, ns], start=(ci == 0), stop=(ci == 1))
                        nc.vector.tensor_scalar_add(Kt[co][:, ns], kp[:],
                                                    vec[co][:, 1:2])
                # Q projection
                for co in range(2):
                    for nch in range(NH // NBLK):
                        ns = slice(nch * NBLK, (nch + 1) * NBLK)
                        qp = psum.tile([128, NBLK], F32, tag="acc", name="acc", bufs=3)
                        for ci in range(2):
                            nc.tensor.matmul(
                                qp[:], wm[ci][:, co * 128:(co + 1) * 128],
                                x1r[ci][:, ns], start=(ci == 0), stop=(ci == 1))
                        nc.vector.tensor_scalar_add(Qt[co][:, ns], qp[:],
                                                    vec[co][:, 0:1])
                # VT projection: VT[m, co] = sum_ci x2[ci, m] v_wT[ci, co]
                for mt in range(MT):
                    ms = slice(mt * 128, (mt + 1) * 128)
                    vp = psum.tile([128, C], F32, tag="acc", name="acc", bufs=3)
                    for ci in range(2):
                        nc.tensor.matmul(vp[:], x2r[ci][:, ms],
                                         wm[ci][:, 2 * C:3 * C],
                                         start=(ci == 0), stop=(ci == 1))
                    nc.vector.tensor_copy(VT[:, mt * C:(mt + 1) * C], vp[:])
                # gate row: x2 columns are pre-permuted so query pixels = 0..NH
                for blk in range(NBLOCKS):
                    ns = slice(blk * NBLK, (blk + 1) * NBLK)
                    gp = psum.tile([1, NBLK], F32, tag="s", name="s", bufs=1)
                    for ci in range(2):
                        nc.tensor.matmul(gp[:], gw[ci][:, 0:1], x1r[ci][:, ns],
                                         start=(ci == 0), stop=False)
                    for ci in range(2):
                        nc.tensor.matmul(gp[:], gw[ci][:, 1:2], x2r[ci][:, ns],
                                         start=False, stop=(ci == 1))
                    nc.scalar.activation(grow[:, ns], gp[:], AF.Sigmoid,
                                         bias=gb[:])

        # ---- main attention loop: logits(j) interleaved with fusion(j-1) ----
        epool = ctx.enter_context(tc.tile_pool(name="epool", bufs=1))
        E = epool.tile([128, MT * NBLK], F32R, tag="E", name="E")

        def fusion_mms(fp, sp, mt):
            es = slice(mt * NBLK, (mt + 1) * NBLK)
            for co in range(2):
                nc.tensor.matmul(
                    fp[co][:], VT[:, mt * C + co * 128: mt * C + (co + 1) * 128],
                    E[:, es], start=(mt == 0), stop=(mt == MT - 1))
            nc.tensor.matmul(sp[:], ones_c[:], E[:, es],
                             start=(mt == 0), stop=(mt == MT - 1))

        def post_block(j, fp, sp):
            ns = slice(j * NBLK, (j + 1) * NBLK)
            with nc.named_scope(f"post{j}"):
                Fs = [work.tile([128, NBLK], F32R, tag=f"Fs{co}", name=f"Fs{co}")
                      for co in range(2)]
                for co in range(2):
                    nc.vector.tensor_copy(Fs[co][:], fp[co][:])
                invs_f = work.tile([1, NBLK], F32, tag="invs_f", name="invs_f")
                nc.vector.reciprocal(invs_f[:], sp[:])
                invs_r = work.tile([1, NBLK], F32R, tag="invs_r", name="invs_r")
                nc.vector.tensor_copy(invs_r[:], invs_f[:])
                bc1 = psum.tile([128, NBLK], F32, tag="acc", name="acc", bufs=3)
                nc.tensor.matmul(bc1[:], ones_k1[:], invs_r[:])
                invs_b = work.tile([128, NBLK], F32, tag="invs_b", name="invs_b")
                nc.vector.tensor_copy(invs_b[:], bc1[:])
                bc2 = psum.tile([128, NBLK], F32, tag="acc", name="acc", bufs=3)
                nc.tensor.matmul(bc2[:], ones_k1[:], grow[:, ns])
                gate_b = work.tile([128, NBLK], F32, tag="gate_b", name="gate_b")
                nc.vector.tensor_copy(gate_b[:], bc2[:])
                for co in range(2):
                    cs = slice(co * 128, (co + 1) * 128)
                    mp = psum.tile([128, NBLK], F32, tag="acc", name="acc", bufs=3)
                    for ci in range(2):
                        nc.tensor.matmul(
                            mp[:], wm[ci][:, 3 * C + co * 128: 3 * C + (co + 1) * 128],
                            Fs[ci][:], start=(ci == 0), stop=(ci == 1))
                    x1t = work.tile([128, NBLK], F32, tag="x1t", name="x1t")
                    nc.sync.dma_start(x1t[:], x1f_d[cs, ns])
                    t1 = work.tile([128, NBLK], F32, tag="t1", name="t1")
                    nc.vector.scalar_tensor_tensor(
                        t1[:], mp[:], vec[co][:, 2:3], invs_b[:],
                        op0=OP.mult, op1=OP.mult)
                    r = work.tile([128, NBLK], F32, tag="r", name="r")
                    nc.scalar.activation(r[:], t1[:], AF.Relu,
                                         bias=vec[co][:, 3:4])
                    rg = work.tile([128, NBLK], F32, tag="rg", name="rg")
                    nc.vector.tensor_mul(rg[:], r[:], gate_b[:])
                    ot = work.tile([128, NBLK], F32, tag="ot", name="ot")
                    nc.vector.tensor_add(ot[:], rg[:], x1t[:])
                    nc.sync.dma_start(out_d[cs, ns], ot[:])

        prev_fp = prev_sp = None
        prev = None
        for blk in range(NBLOCKS):
            ns = slice(blk * NBLK, (blk + 1) * NBLK)
            with nc.named_scope(f"blk{blk}"):
                if prev is not None:
                    prev_sp = psum.tile([1, NBLK], F32, tag="s", name="s", bufs=1)
                    prev_fp = [psum.tile([128, NBLK], F32, tag="acc", name="acc",
                                         bufs=3) for _ in range(2)]
                for mt2 in range(MT // 2):
                    lp = psum.tile([128, 2 * NBLK], F32, tag="L", name="L", bufs=2)
                    for sub in range(2):
                        mt = 2 * mt2 + sub
                        msl = slice(mt * 128, (mt + 1) * 128)
                        for ci in range(2):
                            nc.tensor.matmul(
                                lp[:, sub * NBLK:(sub + 1) * NBLK],
                                Kt[ci][:, msl], Qt[ci][:, ns],
                                start=(ci == 0), stop=(ci == 1))
                    if prev is not None:
                        fusion_mms(prev_fp, prev_sp, 2 * mt2)
                        fusion_mms(prev_fp, prev_sp, 2 * mt2 + 1)
                    nc.scalar.activation(
                        E[:, mt2 * 2 * NBLK:(mt2 + 1) * 2 * NBLK], lp[:],
                        AF.Exp, scale=SCALE)
            if prev is not None:
                post_block(prev, prev_fp, prev_sp)
            prev = blk
        with nc.named_scope("tail"):
            prev_sp = psum.tile([1, NBLK], F32, tag="s", name="s", bufs=1)
            prev_fp = [psum.tile([128, NBLK], F32, tag="acc", name="acc", bufs=3)
                       for _ in range(2)]
            for mt in range(MT):
                fusion_mms(prev_fp, prev_sp, mt)
        post_block(prev, prev_fp, prev_sp)
    nc.compile()
    return nc


_NC = None


def _get_nc():
    global _NC
    if _NC is None:
        _NC = build()
    return _NC


def kernel(**inputs):
    x1 = np.ascontiguousarray(np.asarray(inputs["x1"], dtype=np.float32)).reshape(B, C, N)
    x2 = np.ascontiguousarray(np.asarray(inputs["x2"], dtype=np.float32)).reshape(B, C, N)
    q_w = np.asarray(inputs["q_w"], np.float32)
    k_w = np.asarray(inputs["k_w"], np.float32)
    v_w = np.asarray(inputs["v_w"], np.float32)
    p_w = np.asarray(inputs["proj_w"], np.float32)
    q_b = np.asarray(inputs["q_b"], np.float32)
    k_b = np.asarray(inputs["k_b"], np.float32)
    v_b = np.asarray(inputs["v_b"], np.float32)
    p_b = np.asarray(inputs["proj_b"], np.float32)
    gamma = np.asarray(inputs["bn_gamma"], np.float32)
    beta = np.asarray(inputs["bn_beta"], np.float32)
    mean = np.asarray(inputs["bn_mean"], np.float32)
    var = np.asarray(inputs["bn_var"], np.float32)
    gate_w = np.asarray(inputs["gate_w"], np.float32)
    gate_b = np.asarray(inputs["gate_b"], np.float32)

    wmat = np.ascontiguousarray(
        np.concatenate([q_w.T, k_w.T, v_w.T, p_w.T], axis=1))  # [C, 4C]
    gw = np.ascontiguousarray(np.stack([gate_w[0, :C], gate_w[0, C:]], axis=1))
    G = gamma / np.sqrt(var + EPS)
    Bc = beta + (p_b + p_w @ v_b - mean) * G
    vecs = np.ascontiguousarray(np.stack([q_b, k_b, G, Bc], axis=1))  # [C, 4]
    gb = gate_b.reshape(1, 1)

    in_maps = []
    for core in range(NCORES):
        b, half = divmod(core, 2)
        hq = slice(half * NH, (half + 1) * NH)
        ho = slice((1 - half) * NH, (2 - half) * NH)
        x1q = np.ascontiguousarray(x1[b][:, hq])
        x2p = np.ascontiguousarray(np.concatenate([x2[b][:, hq], x2[b][:, ho]],
                                                  axis=1))
        in_maps.append({
            "x1r": x1q, "x1f": x1q, "x2r": x2p,
            "wmat": wmat, "gw": gw, "vecs": vecs, "gateb": gb,
        })

    nc = _get_nc()
    res = run_bass_kernel_spmd(nc, in_maps, core_ids=list(range(NCORES)))
    out = np.empty((B, C, N), np.float32)
    for core in range(NCORES):
        b, half = divmod(core, 2)
        out[b, :, half * NH:(half + 1) * NH] = res.results[core]["out"]
    return out.reshape(B, C, H, W)
